# revision 32
# baseline (speedup 1.0000x reference)
"""Trainium2 Bass kernel for nn_Attention_71846212928150.

Self-attention block (pre-LN + silu, QKV projections, per-head attention with
q/k LayerNorms, output projection), sharded over 8 NeuronCores by heads:
core c owns heads {2c, 2c+1} = inner columns [128c, 128c+128).

v2 design (vs. the fp32r v1): all PE operands are bf16 (PSUM accumulation
stays fp32), all transposes run on the DMA XBAR (dma_start_transpose) instead
of the PE, q/k LN sums ride the QKV matmul as two host-precomputed row-sum
weight columns, the stats AllReduce is split into two chunks overlapped with
compute, the attention loop is software-pipelined (S(kb+1) issued before
PV(kb)) with double-buffered PSUM so the PE never idles, and the softmax
denominators are batched into a [128, 32] reciprocal instead of a 1-partition
15us DVE reciprocal per (batch, head).
"""

import numpy as np

import concourse.bass as bass
import concourse.mybir as mybir
import concourse.tile as tile

F32 = mybir.dt.float32
BF16 = mybir.dt.bfloat16
FP8 = mybir.dt.float8e4
I32 = mybir.dt.int32
AF = mybir.ActivationFunctionType
ALU = mybir.AluOpType
AX = mybir.AxisListType

B = 2
C = 1024
H = 16
DH = 64
INNER = H * DH
NCORES = 8
HL = H // NCORES          # 2 heads per core
CL = HL * DH              # 128 local inner columns
QKV = 3 * CL              # 384
QKVW = QKV + 2            # + sum_q / sum_k stat columns
KT = C // 128             # 8 contraction tiles over C
EPS = 1e-5
MAGIC = 0x5F3759DF


def _quake_rsqrt(nc, pool, vpe, shape, iters=3, suffix=""):
    """rstd = 1/sqrt(vpe) entirely on DVE (fp32 bitcast + Newton steps)."""
    y = pool.tile(list(shape), F32, name=f"qk_y{suffix}")
    t2 = pool.tile(list(shape), F32, name=f"qk_t2{suffix}")
    nc.vector.tensor_scalar(
        out=y.bitcast(I32), in0=vpe.bitcast(I32), scalar1=1, scalar2=None,
        op0=ALU.logical_shift_right)
    nc.vector.tensor_scalar(
        out=y.bitcast(I32), in0=y.bitcast(I32), scalar1=-1, scalar2=MAGIC,
        op0=ALU.mult, op1=ALU.add)
    for _ in range(iters):
        nc.vector.tensor_tensor(out=t2, in0=y, in1=y, op=ALU.mult)
        nc.vector.tensor_tensor(out=t2, in0=t2, in1=vpe, op=ALU.mult)
        nc.vector.tensor_scalar(out=t2, in0=t2, scalar1=-0.5, scalar2=1.5,
                                op0=ALU.mult, op1=ALU.add)
        nc.vector.tensor_tensor(out=y, in0=y, in1=t2, op=ALU.mult)
    return y


def _fixup_module(nc):
    """Adapt Tile-emitted BIR to this container's walrus build.

    1. The tail `EVENT_SEMAPHORE_RANGE_CLEAR` InstISA (opcode 176) is not
       understood by this walrus' birverifier. Replace it with one
       EventSemaphore sem-write-0 per semaphore in the cleared range.
    2. Drain instructions carrying more than one semaphore wait fail codegen;
       hoist the extra waits into standalone EventSemaphore waits.
    """
    for f in nc.m.functions:
        for bb in f.blocks:
            newlist = []
            changed = False
            for ins in bb.instructions:
                tn = type(ins).__name__
                if tn == "InstISA" and getattr(ins, "isa_opcode", None) == 176:
                    ad = ins.ant_dict or {}
                    first = ad.get("range_first")
                    last = ad.get("range_last")
                    if first is not None and last is not None:
                        si = ins.sync_info
                        sems = list(range(first, last + 1))
                        for k, sem in enumerate(sems):
                            ev = mybir.InstEventSemaphore(
                                name=f"{ins.name}-clr{k}", engine=ins.engine,
                                ins=[], outs=[])
                            upd = mybir.SyncUpdate(
                                sync_type="semaphore", id=sem,
                                update_mode="sem-wr-imm", update_value=0)
                            on_wait = (list(si.on_wait)
                                       if (k == 0 and si is not None and si.on_wait)
                                       else [])
                            ev.sync_info = mybir.SyncInfo(
                                on_wait=on_wait, on_update=[upd])
                            newlist.append(ev)
                        if si is not None and si.on_update:
                            evf = mybir.InstEventSemaphore(
                                name=f"{ins.name}-clrf", engine=ins.engine,
                                ins=[], outs=[])
                            evf.sync_info = mybir.SyncInfo(
                                on_wait=[], on_update=list(si.on_update))
                            newlist.append(evf)
                    changed = True
                    continue
                si = ins.sync_info
                if (si is not None and si.on_wait is not None
                        and len(si.on_wait) > 1):
                    waits = list(si.on_wait)
                    for i, w in enumerate(waits[1:]):
                        ev = mybir.InstEventSemaphore(
                            name=f"{ins.name}-hw{i}", engine=ins.engine,
                            ins=[], outs=[])
                        ev.sync_info = mybir.SyncInfo(on_wait=[w], on_update=[])
                        newlist.append(ev)
                    si.on_wait = [waits[0]]
                    ins.sync_info = si
                    changed = True
                newlist.append(ins)
            if changed:
                bb.instructions = newlist
    return nc


def build_bass(n_tok_per_batch, n_cores=NCORES):
    N = n_tok_per_batch
    T = B * N
    NT = T // 128             # token tiles (32)
    KB = N // 128             # key tiles per batch (16)

    nc = bass.Bass(trn_type="TRN2", num_devices=n_cores)

    x = nc.dram_tensor("x", [T, C], BF16, kind="ExternalInput")
    w_all = nc.dram_tensor("w_all", [C, QKVW], BF16, kind="ExternalInput")
    b_all = nc.dram_tensor("b_all", [1, QKVW], BF16, kind="ExternalInput")
    gbe = nc.dram_tensor("gbe", [128, 4], F32, kind="ExternalInput")
    w_o_loc = nc.dram_tensor("w_o_loc", [CL, C], BF16, kind="ExternalInput")
    out_t = nc.dram_tensor("out_t", [C, T], BF16, kind="ExternalOutput")

    with tile.TileContext(nc) as tc:
        _body(tc, x, w_all, b_all, gbe, w_o_loc, out_t,
              N=N, T=T, NT=NT, KB=KB, n_cores=n_cores)
    return _fixup_module(nc)


def _body(tc, x, w_all, b_all, gbe, w_o_loc, out_t, N, T, NT, KB, n_cores):
    nc = tc.nc

    from contextlib import ExitStack
    octx = ExitStack()
    persist = octx.enter_context(tc.tile_pool(name="persist", bufs=1))

    GB = 4                       # token tiles per phase-1 group
    NG = NT // GB                # 8 groups
    NCH = 2                      # AllReduce chunks (chunk == batch)
    TCH = NT // NCH              # 16 tiles per chunk

    w_all_sb = persist.tile([128, KT, QKVW], BF16)
    for kt in range(KT):
        nc.scalar.dma_start(out=w_all_sb[:, kt, :],
                          in_=w_all[kt * 128:(kt + 1) * 128, :])
    b_row = persist.tile([1, QKVW], BF16)
    nc.scalar.dma_start(out=b_row, in_=b_all[0:1, :])
    ones_1p = persist.tile([1, 128], BF16)
    nc.vector.memset(ones_1p, 1.0)
    gbe_sb = persist.tile([128, 4], F32)
    nc.scalar.dma_start(out=gbe_sb, in_=gbe[:, :])
    w_o_sb = persist.tile([128, C], BF16)
    nc.scalar.dma_start(out=w_o_sb, in_=w_o_loc[:, :])

    qT = persist.tile([128, T], BF16)       # [local col, token]
    kTt = persist.tile([128, T], BF16)
    v_aug = persist.tile([128, NT, 144], FP8)  # [tok%128, tile, 2x(64 v + 1 + pad)]
    qk_pre = persist.tile([128, NT, 256], BF16)  # [tok%128, tile, q|k col]
    # stats cols: 0=sum_q, 1=sum_k, 2=ssq_q, 3=ssq_k
    stats = persist.tile([128, NCH, TCH, 4], F32)
    stats_all = persist.tile([128, NCH, TCH, 4], F32)
    o_un = persist.tile([128, 2 * B * HL, 1024], BF16)  # [dim(65), slot, qtok]
    onorm = persist.tile([128, T], BF16)
    siluo = persist.tile([128, T], BF16)
    scr = persist.tile([128, 128], BF16)

    ones_col = persist.tile([128, NT], F32)
    nc.vector.memset(ones_col, 1.0)
    nc.vector.tensor_copy(out=v_aug[:, :, 64:65], in_=ones_col)
    nc.vector.tensor_copy(out=v_aug[:, :, 136:137], in_=ones_col)

    dram = octx.enter_context(tc.tile_pool(name="dram", bufs=1, space="DRAM"))
    cc_in = [dram.tile([128, TCH * 4], F32, name=f"cc_in{c}")
             for c in range(NCH)]
    cc_out = [dram.tile([128, TCH * 4], F32, name=f"cc_out{c}",
                        addr_space="Shared")
              for c in range(NCH)]

    ph1 = octx.enter_context(tc.tile_pool(name="ph1", bufs=3))
    ph1t = octx.enter_context(tc.tile_pool(name="ph1t", bufs=8))
    ph1s = octx.enter_context(tc.tile_pool(name="ph1s", bufs=4))
    ph2 = octx.enter_context(tc.tile_pool(name="ph2", bufs=1))
    ph3 = octx.enter_context(tc.tile_pool(name="ph3", bufs=2))

    pctx = ExitStack()           # phase-1 PSUM, closed before attention PSUM
    ph1q = pctx.enter_context(tc.tile_pool(name="ph1q", bufs=3, space="PSUM"))

    # ---------------- phase 1: x LN+silu, XBAR transpose, QKV ----------------
    # Split into a stats part (no PE dependency) and a compute part, emitted
    # one group ahead, so the DVE queue prefetches bn_stats instead of
    # head-of-line blocking on matmul-dependent evictions.
    def phase1_stats(g):
        xg = ph1.tile([128, GB, C], BF16, name="xg")
        nc.scalar.dma_start(
            out=xg,
            in_=x[g * GB * 128:(g + 1) * GB * 128, :].rearrange(
                "(t p) c -> p t c", p=128))

        stats6 = ph1s.tile([128, GB, 2, 6], F32, name="stats6")
        for t in range(GB):
            for h2 in range(2):
                nc.vector.bn_stats(out=stats6[:, t, h2, :],
                                   in_=xg[:, t, h2 * 512:(h2 + 1) * 512])
        mv = ph1s.tile([128, GB, 2], F32, name="mv")
        for t in range(GB):
            nc.vector.bn_aggr(out=mv[:, t, :], in_=stats6[:, t, :, :])

        vpe = ph1s.tile([128, GB, 1], F32, name="vpe")
        nc.vector.tensor_scalar(out=vpe, in0=mv[:, :, 1:2], scalar1=EPS,
                                scalar2=None, op0=ALU.add)
        rstd = _quake_rsqrt(nc, ph1s, vpe, (128, GB, 1), iters=2, suffix="x")
        nmr = ph1s.tile([128, GB, 1], F32, name="nmr")
        nc.vector.tensor_tensor(out=nmr, in0=mv[:, :, 0:1], in1=rstd,
                                op=ALU.mult)
        nc.vector.tensor_scalar(out=nmr, in0=nmr, scalar1=-1.0,
                                scalar2=None, op0=ALU.mult)
        # silu(LN(x)) + x^T XBAR here (no PE dependency), so the next group's
        # transposed input is ready before this group's matmuls retire and the
        # PE never waits at a group boundary
        xsTs = []
        for t in range(GB):
            nc.scalar.activation(out=xg[:, t, :], in_=xg[:, t, :],
                                 func=AF.Silu,
                                 bias=nmr[:, t, :],
                                 scale=rstd[:, t, :])
            # [tok, 1024] -> [ch%128, ch//128, tok]; ACT-issued for the back
            # half: the in-flight stats AllReduce freezes SP-queue DMAs
            xsT = ph1t.tile([128, KT, 128], BF16, name="xsT")
            eng = nc.scalar if g >= 4 else nc.sync
            eng.dma_start_transpose(out=xsT, in_=xg[:, t, :])
            xsTs.append(xsT)
        return xsTs

    def phase1_compute(g, pre):
        xsTs = pre
        for t in range(GB):
            tt = g * GB + t
            ch = tt // TCH
            ti = tt % TCH
            xsT = xsTs[t]
            pqkv = ph1q.tile([128, QKVW], F32, name="pqkv")
            for kt in range(KT):
                nc.tensor.matmul(
                    pqkv,
                    lhsT=xsT[:, kt, :],
                    rhs=w_all_sb[:, kt, :],
                    start=(kt == 0), stop=False)
            # bias (and bias-sum stat constants) as a rank-1 accumulation
            nc.tensor.matmul(pqkv, lhsT=ones_1p, rhs=b_row,
                             start=False, stop=True)

            # evictions (PSUM fp32 -> SBUF bf16/fp8); bias already added
            nc.scalar.copy(out=qk_pre[:, tt, :], in_=pqkv[:, 0:256])
            nc.vector.tensor_copy(
                out=v_aug[:, tt, :].rearrange("p (h e) -> p h e", e=72)[:, :, 0:64],
                in_=pqkv[:, 256:384].rearrange("p (h e) -> p h e", e=64))
            # q/k sums rode the matmul in the 2 extra weight columns
            nc.vector.tensor_copy(out=stats[:, ch, ti, 0:2],
                                  in_=pqkv[:, QKV:QKV + 2])
            # sums of squares on the otherwise idle Pool engine
            sq = ph1s.tile([128, 2, 128], F32, name="sq")
            nc.gpsimd.tensor_tensor(
                out=sq.rearrange("p a b -> p (a b)"), in0=qk_pre[:, tt, :],
                in1=qk_pre[:, tt, :], op=ALU.mult)
            nc.vector.tensor_reduce(out=stats[:, ch, ti, 2:4], in_=sq,
                                    axis=AX.X, op=ALU.add)

    def emit_allreduce(ch):
        nc.scalar.dma_start(out=cc_in[ch],
                            in_=stats[:, ch].rearrange("p a b -> p (a b)"))
        nc.gpsimd.collective_compute(
            "AllReduce", ALU.add,
            replica_groups=[list(range(n_cores))],
            ins=[cc_in[ch].opt()], outs=[cc_out[ch].opt()])
        nc.sync.dma_start(
            out=stats_all[:, ch].rearrange("p a b -> p (a b)"),
            in_=cc_out[ch])

    # phase 2+3 for one chunk: full-inner LN stats -> normalize -> transpose
    def phase23_chunk(ch):
        qk_sn = []
        for which in range(2):  # 0 -> q, 1 -> k
            s_sum = stats_all[:, ch, :, which]
            s_ssq = stats_all[:, ch, :, 2 + which]
            m = ph2.tile([128, TCH], F32, name=f"m_{ch}_{which}")
            nc.vector.tensor_scalar(out=m, in0=s_sum, scalar1=1.0 / INNER,
                                    scalar2=None, op0=ALU.mult)
            msq = ph2.tile([128, TCH], F32, name=f"msq_{ch}_{which}")
            nc.vector.tensor_scalar(out=msq, in0=s_ssq, scalar1=1.0 / INNER,
                                    scalar2=None, op0=ALU.mult)
            tmp = ph2.tile([128, TCH], F32, name=f"tmp_{ch}_{which}")
            nc.vector.tensor_tensor(out=tmp, in0=m, in1=m, op=ALU.mult)
            nc.vector.tensor_tensor(out=tmp, in0=msq, in1=tmp, op=ALU.subtract)
            nc.vector.tensor_scalar(out=tmp, in0=tmp, scalar1=EPS,
                                    scalar2=None, op0=ALU.add)
            rstd = _quake_rsqrt(nc, ph2, tmp, (128, TCH),
                                suffix=f"_{ch}_{which}")
            nmr = ph2.tile([128, TCH], F32, name=f"nmr_{ch}_{which}")
            nc.vector.tensor_tensor(out=nmr, in0=m, in1=rstd, op=ALU.mult)
            nc.vector.tensor_scalar(out=nmr, in0=nmr, scalar1=-1.0,
                                    scalar2=None, op0=ALU.mult)
            qk_sn.append((m, rstd, nmr))

        (mq, rq, nq) = qk_sn[0]
        (mk, rk, nk) = qk_sn[1]
        # normalize into one [128, TCH, 128] staging tile per tensor, then a
        # SINGLE chunk-wide XBAR transpose each ([tok, (tile col)] ->
        # [col, (tile tok)]), then one chunk-wide gain pass
        qnc = ph3.tile([128, TCH, 128], BF16, name="qnc")
        knc = ph3.tile([128, TCH, 128], BF16, name="knc")
        for ti in range(TCH):
            tt = ch * TCH + ti
            nc.vector.tensor_scalar(
                out=qnc[:, ti, :], in0=qk_pre[:, tt, 0:128],
                scalar1=mq[:, ti:ti + 1], scalar2=rq[:, ti:ti + 1],
                op0=ALU.subtract, op1=ALU.mult)
            nc.scalar.activation(
                out=knc[:, ti, :], in_=qk_pre[:, tt, 128:256],
                func=AF.Identity,
                bias=nk[:, ti:ti + 1], scale=rk[:, ti:ti + 1])
        lo, hi = ch * TCH * 128, (ch + 1) * TCH * 128
        teng = nc.scalar if ch == 0 else nc.sync
        teng.dma_start_transpose(
            out=qT[:, lo:hi].rearrange("p (a b) -> p a b", a=TCH),
            in_=qnc[:, :, :].rearrange("p a b -> p (a b)"))
        teng.dma_start_transpose(
            out=kTt[:, lo:hi].rearrange("p (a b) -> p a b", a=TCH),
            in_=knc[:, :, :].rearrange("p a b -> p (a b)"))
        nc.vector.tensor_scalar(
            out=qT[:, lo:hi], in0=qT[:, lo:hi],
            scalar1=gbe_sb[:, 0:1], scalar2=gbe_sb[:, 1:2],
            op0=ALU.mult, op1=ALU.add)
        nc.scalar.activation(
            out=kTt[:, lo:hi], in_=kTt[:, lo:hi], func=AF.Identity,
            bias=gbe_sb[:, 3:4], scale=gbe_sb[:, 2:3])

    # ---------------- phase 4: attention ----------------
    att = octx.enter_context(tc.tile_pool(name="att", bufs=3))
    dramsc = octx.enter_context(tc.tile_pool(name="dramsc", bufs=2,
                                             space="DRAM"))
    dnp = octx.enter_context(tc.tile_pool(name="dnp", bufs=2))
    actx = ExitStack()           # attention PSUM, closed before phase-5 PSUM

    NPAIR = KB // 2
    DR = mybir.MatmulPerfMode.DoubleRow

    def attention_g(b, h, g, attp, attpo, filler=None):
        # one q-chunk group: tokens [g*1024, (g+1)*1024) of batch b, head h.
        # filler: list of thunks emitting one PE op each (out-projection
        # matmuls), drained one per kb to fill the exp-wait bubbles.
        slot = b * 4 + h * 2 + g

        def emit_pv(pO, eS2, pair):
            # fp8 DoubleRow PV: contract 256 keys (2 kb tiles) per matmul
            # at 0.5 cycles/row
            vt0 = b * KB + 2 * pair
            for qi in range(2):
                nc.tensor.matmul(
                    pO[0:65, qi * 512:(qi + 1) * 512],
                    lhsT=v_aug[:, vt0:vt0 + 2, h * 72:h * 72 + 65],
                    rhs=eS2[:, :, qi * 512:(qi + 1) * 512],
                    start=(pair == 0), stop=(pair == NPAIR - 1),
                    perf_mode=DR)

        pO = attpo.tile([128, 1024], F32, name="pO", tag="pO")
        pend = None          # software pipeline: delay PV by one kb pair
        eS2 = None
        for kb in range(KB):
            pS = attp.tile([128, 1024], F32, name="pS", tag="pS")
            for qi in range(2):
                q0 = b * N + g * 1024 + qi * 512
                nc.tensor.matmul(
                    pS[:, qi * 512:(qi + 1) * 512],
                    lhsT=kTt[h * 64:(h + 1) * 64,
                             b * N + kb * 128:b * N + (kb + 1) * 128],
                    rhs=qT[h * 64:(h + 1) * 64, q0:q0 + 512],
                    start=True, stop=True)
            if pend is not None and kb % 2 == 0:
                emit_pv(pO, *pend)
                pend = None
            if filler:
                filler.pop(0)()
            if kb % 2 == 0:
                eS2 = att.tile([128, 2, 1024], FP8, name="eS2")
            nc.scalar.activation(out=eS2[:, kb % 2, :], in_=pS,
                                 func=AF.Exp)
            if kb % 2 == 1:
                pend = (eS2, kb // 2)
        emit_pv(pO, *pend)
        # evict unnormalized O + raw denominator row
        nc.vector.tensor_copy(out=o_un[0:65, slot, :], in_=pO[0:65, :])

    def denorm_batch(b):
        # batch b's denominators live in o_un[64, b*4:(b+1)*4, :]
        dn_dram = dramsc.tile([1, 4096], BF16, name="dn_dram")
        nc.sync.dma_start(
            out=dn_dram,
            in_=o_un[64:65, b * 4:(b + 1) * 4, :].rearrange(
                "p a t -> p (a t)"))
        dn_g = dnp.tile([128, 32], BF16, name="dn_g")
        nc.sync.dma_start(
            out=dn_g,
            in_=dn_dram[0:1, :].rearrange("o (p c) -> (o p) c", p=128))
        rdn = dnp.tile([128, 32], BF16, name="rdn")
        with nc.allow_low_precision(reason="softmax denom reciprocal, 2e-2 budget"):
            nc.vector.reciprocal(out=rdn, in_=dn_g)
        rdn_dram = dramsc.tile([1, 4096], BF16, name="rdn_dram")
        nc.sync.dma_start(
            out=rdn_dram[0:1, :].rearrange("o (p c) -> (o p) c", p=128),
            in_=rdn)
        dnb = dnp.tile([64, 4096], BF16, name="dnb")
        nc.sync.dma_start(out=dnb, in_=rdn_dram.to_broadcast([64, 4096]))
        for h in range(HL):
            for g in range(2):
                slot = b * 4 + h * 2 + g
                sg = h * 2 + g
                nc.vector.tensor_tensor(
                    out=onorm[h * 64:(h + 1) * 64,
                              b * N + g * 1024:b * N + (g + 1) * 1024],
                    in0=o_un[0:64, slot, :],
                    in1=dnb[:, sg * 1024:(sg + 1) * 1024],
                    op=ALU.mult)

    def silu_batch(b):
        nc.scalar.activation(out=siluo[:, b * N:(b + 1) * N],
                             in_=onorm[:, b * N:(b + 1) * N], func=AF.Silu)

    # ---------------- emission schedule ----------------
    pre = phase1_stats(0)
    for g in range(NG):
        nxt = phase1_stats(g + 1) if g + 1 < NG else None
        phase1_compute(g, pre)
        pre = nxt
        if g == 3:
            emit_allreduce(0)
    # ph23(0)'s XBAR transposes freeze while a collective is in flight, so
    # launch the second AllReduce only after they are issued
    phase23_chunk(0)
    emit_allreduce(1)
    pctx.close()                 # free phase-1 PSUM banks
    attp = actx.enter_context(tc.tile_pool(name="attp", bufs=2, space="PSUM"))
    attpo = actx.enter_context(tc.tile_pool(name="attpo", bufs=1,
                                            space="PSUM"))
    ph5p = actx.enter_context(tc.tile_pool(name="ph5p", bufs=2, space="PSUM"))
    ph5 = octx.enter_context(tc.tile_pool(name="ph5", bufs=4))

    def make_ph5_op(tk, ct, on_act=False):
        def emit():
            po = ph5p.tile([128, 512], F32, name="po")
            nc.tensor.matmul(
                po,
                lhsT=w_o_sb[:, ct * 128:(ct + 1) * 128],
                rhs=siluo[:, tk * 512:(tk + 1) * 512],
                start=True, stop=True)
            ev = ph5.tile([128, 512], BF16, name="ev")
            if on_act:
                nc.scalar.copy(out=ev, in_=po)
            else:
                nc.vector.tensor_copy(out=ev, in_=po)
            nc.sync.dma_start(
                out=out_t[ct * 128:(ct + 1) * 128,
                          tk * 512:(tk + 1) * 512],
                in_=ev)
        return emit

    attention_g(0, 0, 0, attp, attpo)
    attention_g(0, 0, 1, attp, attpo)
    attention_g(0, 1, 0, attp, attpo)
    phase23_chunk(1)
    attention_g(0, 1, 1, attp, attpo)
    denorm_batch(0)
    attention_g(1, 0, 0, attp, attpo)
    attention_g(1, 0, 1, attp, attpo)
    silu_batch(0)
    # batch-0 out-projection interleaved into batch-1 attention: one matmul
    # per kb fills the PE bubble left by the exp dependency
    fill = [make_ph5_op(tk, ct) for tk in range(4) for ct in range(KT)]
    attention_g(1, 1, 0, attp, attpo, filler=fill)
    attention_g(1, 1, 1, attp, attpo, filler=fill)
    assert not fill
    denorm_batch(1)
    silu_batch(1)
    for tk in range(4, 8):
        for ct in range(KT):
            make_ph5_op(tk, ct, on_act=(ct % 2 == 1))()
    actx.close()                 # free attention PSUM banks

    octx.close()


def make_in_maps(inputs, n_tok_per_batch, n_cores=NCORES):
    """Slice full inputs into per-core input maps (head sharding)."""
    import ml_dtypes
    bf16 = ml_dtypes.bfloat16

    x = np.ascontiguousarray(np.asarray(inputs["x"], np.float32)
                             .reshape(B * n_tok_per_batch, C)).astype(bf16)
    w_q = np.asarray(inputs["w_q"], np.float32)
    w_k = np.asarray(inputs["w_k"], np.float32)
    w_v = np.asarray(inputs["w_v"], np.float32)
    b_q = np.asarray(inputs["b_q"], np.float32)
    b_k = np.asarray(inputs["b_k"], np.float32)
    b_v = np.asarray(inputs["b_v"], np.float32)
    g_q = np.asarray(inputs["g_q"], np.float32)
    be_q = np.asarray(inputs["be_q"], np.float32)
    g_k = np.asarray(inputs["g_k"], np.float32)
    be_k = np.asarray(inputs["be_k"], np.float32)
    w_o = np.asarray(inputs["w_o"], np.float32)

    scale = float(INNER) ** -0.5
    in_maps = []
    for c in range(n_cores):
        cols = slice(c * CL, (c + 1) * CL)
        wq_l = w_q[:, cols]
        wk_l = w_k[:, cols]
        wv_l = w_v[:, cols]
        w_all = np.ascontiguousarray(np.concatenate(
            [wq_l, wk_l, wv_l,
             wq_l.sum(axis=1, keepdims=True),
             wk_l.sum(axis=1, keepdims=True)], axis=1)).astype(bf16)
        b_all = np.ascontiguousarray(
            np.concatenate([b_q[cols], b_k[cols], b_v[cols],
                            [b_q[cols].sum()], [b_k[cols].sum()]])[None, :]
        ).astype(bf16)
        gbe = np.ascontiguousarray(
            np.stack([g_q[cols] * scale, be_q[cols] * scale,
                      g_k[cols], be_k[cols]], axis=1))
        w_o_c = np.ascontiguousarray(w_o[cols, :]).astype(bf16)
        in_maps.append({
            "x": x, "w_all": w_all, "b_all": b_all,
            "gbe": gbe, "w_o_loc": w_o_c,
        })
    return in_maps


def combine_outputs(out_ts, inputs, n_tok_per_batch):
    b_o = np.asarray(inputs["b_o"], np.float32)
    acc = np.zeros(out_ts[0].shape, dtype=np.float32)
    for o in out_ts:
        acc += np.asarray(o, dtype=np.float32)
    out = acc.T + b_o[None, :]
    return out.reshape(B, n_tok_per_batch, C).astype(np.float32)


_NC_CACHE = {}


def kernel(**inputs):
    from concourse.bass_utils import run_bass_kernel_spmd

    n_tok = np.asarray(inputs["x"]).shape[1]
    if n_tok not in _NC_CACHE:
        _NC_CACHE[n_tok] = build_bass(n_tok)
    nc = _NC_CACHE[n_tok]
    in_maps = make_in_maps(inputs, n_tok)
    res = run_bass_kernel_spmd(nc, in_maps, core_ids=list(range(NCORES)))
    out_ts = [r["out_t"] for r in res.results]
    return combine_outputs(out_ts, inputs, n_tok)


# revision 33
# speedup vs baseline: 1.0649x; 1.0649x over previous
"""Trainium2 Bass kernel for nn_Attention_71846212928150.

Self-attention block (pre-LN + silu, QKV projections, per-head attention with
q/k LayerNorms, output projection), sharded over 8 NeuronCores by heads:
core c owns heads {2c, 2c+1} = inner columns [128c, 128c+128).

v2 design (vs. the fp32r v1): all PE operands are bf16 (PSUM accumulation
stays fp32), all transposes run on the DMA XBAR (dma_start_transpose) instead
of the PE, q/k LN sums ride the QKV matmul as two host-precomputed row-sum
weight columns, the stats AllReduce is split into two chunks overlapped with
compute, the attention loop is software-pipelined (S(kb+1) issued before
PV(kb)) with double-buffered PSUM so the PE never idles, and the softmax
denominators are batched into a [128, 32] reciprocal instead of a 1-partition
15us DVE reciprocal per (batch, head).
"""

import numpy as np

import concourse.bass as bass
import concourse.mybir as mybir
import concourse.tile as tile
from concourse.masks import make_identity

F32 = mybir.dt.float32
BF16 = mybir.dt.bfloat16
FP8 = mybir.dt.float8e4
I32 = mybir.dt.int32
AF = mybir.ActivationFunctionType
ALU = mybir.AluOpType
AX = mybir.AxisListType

B = 2
C = 1024
H = 16
DH = 64
INNER = H * DH
NCORES = 8
HL = H // NCORES          # 2 heads per core
CL = HL * DH              # 128 local inner columns
QKV = 3 * CL              # 384
QKVW = QKV + 2            # + sum_q / sum_k stat columns
KT = C // 128             # 8 contraction tiles over C
EPS = 1e-5
MAGIC = 0x5F3759DF


def _quake_rsqrt(nc, pool, vpe, shape, iters=3, suffix=""):
    """rstd = 1/sqrt(vpe) entirely on DVE (fp32 bitcast + Newton steps)."""
    y = pool.tile(list(shape), F32, name=f"qk_y{suffix}")
    t2 = pool.tile(list(shape), F32, name=f"qk_t2{suffix}")
    nc.vector.tensor_scalar(
        out=y.bitcast(I32), in0=vpe.bitcast(I32), scalar1=1, scalar2=None,
        op0=ALU.logical_shift_right)
    nc.vector.tensor_scalar(
        out=y.bitcast(I32), in0=y.bitcast(I32), scalar1=-1, scalar2=MAGIC,
        op0=ALU.mult, op1=ALU.add)
    for _ in range(iters):
        nc.vector.tensor_tensor(out=t2, in0=y, in1=y, op=ALU.mult)
        nc.vector.tensor_tensor(out=t2, in0=t2, in1=vpe, op=ALU.mult)
        nc.vector.tensor_scalar(out=t2, in0=t2, scalar1=-0.5, scalar2=1.5,
                                op0=ALU.mult, op1=ALU.add)
        nc.vector.tensor_tensor(out=y, in0=y, in1=t2, op=ALU.mult)
    return y


def _fixup_module(nc):
    """Adapt Tile-emitted BIR to this container's walrus build.

    1. The tail `EVENT_SEMAPHORE_RANGE_CLEAR` InstISA (opcode 176) is not
       understood by this walrus' birverifier. Replace it with one
       EventSemaphore sem-write-0 per semaphore in the cleared range.
    2. Drain instructions carrying more than one semaphore wait fail codegen;
       hoist the extra waits into standalone EventSemaphore waits.
    """
    for f in nc.m.functions:
        for bb in f.blocks:
            newlist = []
            changed = False
            for ins in bb.instructions:
                tn = type(ins).__name__
                if tn == "InstISA" and getattr(ins, "isa_opcode", None) == 176:
                    ad = ins.ant_dict or {}
                    first = ad.get("range_first")
                    last = ad.get("range_last")
                    if first is not None and last is not None:
                        si = ins.sync_info
                        sems = list(range(first, last + 1))
                        for k, sem in enumerate(sems):
                            ev = mybir.InstEventSemaphore(
                                name=f"{ins.name}-clr{k}", engine=ins.engine,
                                ins=[], outs=[])
                            upd = mybir.SyncUpdate(
                                sync_type="semaphore", id=sem,
                                update_mode="sem-wr-imm", update_value=0)
                            on_wait = (list(si.on_wait)
                                       if (k == 0 and si is not None and si.on_wait)
                                       else [])
                            ev.sync_info = mybir.SyncInfo(
                                on_wait=on_wait, on_update=[upd])
                            newlist.append(ev)
                        if si is not None and si.on_update:
                            evf = mybir.InstEventSemaphore(
                                name=f"{ins.name}-clrf", engine=ins.engine,
                                ins=[], outs=[])
                            evf.sync_info = mybir.SyncInfo(
                                on_wait=[], on_update=list(si.on_update))
                            newlist.append(evf)
                    changed = True
                    continue
                si = ins.sync_info
                if (si is not None and si.on_wait is not None
                        and len(si.on_wait) > 1):
                    waits = list(si.on_wait)
                    for i, w in enumerate(waits[1:]):
                        ev = mybir.InstEventSemaphore(
                            name=f"{ins.name}-hw{i}", engine=ins.engine,
                            ins=[], outs=[])
                        ev.sync_info = mybir.SyncInfo(on_wait=[w], on_update=[])
                        newlist.append(ev)
                    si.on_wait = [waits[0]]
                    ins.sync_info = si
                    changed = True
                newlist.append(ins)
            if changed:
                bb.instructions = newlist
    return nc


def build_bass(n_tok_per_batch, n_cores=NCORES):
    N = n_tok_per_batch
    T = B * N
    NT = T // 128             # token tiles (32)
    KB = N // 128             # key tiles per batch (16)

    nc = bass.Bass(trn_type="TRN2", num_devices=n_cores)

    x = nc.dram_tensor("x", [T, C], BF16, kind="ExternalInput")
    w_all = nc.dram_tensor("w_all", [C, QKVW], BF16, kind="ExternalInput")
    b_all = nc.dram_tensor("b_all", [1, QKVW], BF16, kind="ExternalInput")
    gbe = nc.dram_tensor("gbe", [128, 4], F32, kind="ExternalInput")
    w_o_loc = nc.dram_tensor("w_o_loc", [CL, C], BF16, kind="ExternalInput")
    out_t = nc.dram_tensor("out_t", [C, T], BF16, kind="ExternalOutput")

    with tile.TileContext(nc) as tc:
        _body(tc, x, w_all, b_all, gbe, w_o_loc, out_t,
              N=N, T=T, NT=NT, KB=KB, n_cores=n_cores)
    return _fixup_module(nc)


def _body(tc, x, w_all, b_all, gbe, w_o_loc, out_t, N, T, NT, KB, n_cores):
    nc = tc.nc

    from contextlib import ExitStack
    octx = ExitStack()
    persist = octx.enter_context(tc.tile_pool(name="persist", bufs=1))

    GB = 4                       # token tiles per phase-1 group
    NG = NT // GB                # 8 groups
    NCH = 2                      # AllReduce chunks (chunk == batch)
    TCH = NT // NCH              # 16 tiles per chunk

    ident = persist.tile([128, 128], BF16)
    make_identity(nc, ident)

    w_all_sb = persist.tile([128, KT, QKVW], BF16)
    for kt in range(KT):
        nc.scalar.dma_start(out=w_all_sb[:, kt, :],
                          in_=w_all[kt * 128:(kt + 1) * 128, :])
    b_row = persist.tile([1, QKVW], BF16)
    nc.scalar.dma_start(out=b_row, in_=b_all[0:1, :])
    ones_1p = persist.tile([1, 128], BF16)
    nc.vector.memset(ones_1p, 1.0)
    gbe_sb = persist.tile([128, 4], F32)
    nc.scalar.dma_start(out=gbe_sb, in_=gbe[:, :])
    w_o_sb = persist.tile([128, C], BF16)
    nc.scalar.dma_start(out=w_o_sb, in_=w_o_loc[:, :])

    qT = persist.tile([128, T], BF16)       # [local col, token]
    kTt = persist.tile([128, T], BF16)
    v_aug = persist.tile([128, NT, 144], FP8)  # [tok%128, tile, 2x(64 v + 1 + pad)]
    qk_pre = persist.tile([128, NT, 256], BF16)  # [tok%128, tile, q|k col]
    # stats cols: 0=sum_q, 1=sum_k, 2=ssq_q, 3=ssq_k
    stats = persist.tile([128, NCH, TCH, 4], F32)
    stats_all = persist.tile([128, NCH, TCH, 4], F32)
    o_un = persist.tile([128, 2 * B * HL, 1024], BF16)  # [dim(65), slot, qtok]
    onorm = persist.tile([128, T], BF16)
    siluo = persist.tile([128, T], BF16)
    scr = persist.tile([128, 128], BF16)

    ones_col = persist.tile([128, NT], F32)
    nc.vector.memset(ones_col, 1.0)
    nc.vector.tensor_copy(out=v_aug[:, :, 64:65], in_=ones_col)
    nc.vector.tensor_copy(out=v_aug[:, :, 136:137], in_=ones_col)

    dram = octx.enter_context(tc.tile_pool(name="dram", bufs=1, space="DRAM"))
    cc_in = [dram.tile([128, TCH * 4], F32, name=f"cc_in{c}")
             for c in range(NCH)]
    cc_out = [dram.tile([128, TCH * 4], F32, name=f"cc_out{c}",
                        addr_space="Shared")
              for c in range(NCH)]

    ph1 = octx.enter_context(tc.tile_pool(name="ph1", bufs=3))
    ph1t = octx.enter_context(tc.tile_pool(name="ph1t", bufs=8))
    ph1s = octx.enter_context(tc.tile_pool(name="ph1s", bufs=4))
    ph2 = octx.enter_context(tc.tile_pool(name="ph2", bufs=1))
    ph3 = octx.enter_context(tc.tile_pool(name="ph3", bufs=2))

    pctx = ExitStack()           # phase-1 PSUM, closed before attention PSUM
    ph1q = pctx.enter_context(tc.tile_pool(name="ph1q", bufs=3, space="PSUM"))
    ph1tp = pctx.enter_context(tc.tile_pool(name="ph1tp", bufs=2, space="PSUM"))

    # ---------------- phase 1: x LN+silu, XBAR transpose, QKV ----------------
    # Split into a stats part (no PE dependency) and a compute part, emitted
    # one group ahead, so the DVE queue prefetches bn_stats instead of
    # head-of-line blocking on matmul-dependent evictions.
    def phase1_stats(g):
        xg = ph1.tile([128, GB, C], BF16, name="xg")
        nc.scalar.dma_start(
            out=xg,
            in_=x[g * GB * 128:(g + 1) * GB * 128, :].rearrange(
                "(t p) c -> p t c", p=128))

        stats6 = ph1s.tile([128, GB, 2, 6], F32, name="stats6")
        for t in range(GB):
            for h2 in range(2):
                nc.vector.bn_stats(out=stats6[:, t, h2, :],
                                   in_=xg[:, t, h2 * 512:(h2 + 1) * 512])
        mv = ph1s.tile([128, GB, 2], F32, name="mv")
        for t in range(GB):
            nc.vector.bn_aggr(out=mv[:, t, :], in_=stats6[:, t, :, :])

        vpe = ph1s.tile([128, GB, 1], F32, name="vpe")
        nc.vector.tensor_scalar(out=vpe, in0=mv[:, :, 1:2], scalar1=EPS,
                                scalar2=None, op0=ALU.add)
        rstd = _quake_rsqrt(nc, ph1s, vpe, (128, GB, 1), iters=2, suffix="x")
        nmr = ph1s.tile([128, GB, 1], F32, name="nmr")
        nc.vector.tensor_tensor(out=nmr, in0=mv[:, :, 0:1], in1=rstd,
                                op=ALU.mult)
        nc.vector.tensor_scalar(out=nmr, in0=nmr, scalar1=-1.0,
                                scalar2=None, op0=ALU.mult)
        # silu(LN(x)) + x^T XBAR here (no PE dependency), so the next group's
        # transposed input is ready before this group's matmuls retire and the
        # PE never waits at a group boundary
        xsTs = []
        for t in range(GB):
            nc.scalar.activation(out=xg[:, t, :], in_=xg[:, t, :],
                                 func=AF.Silu,
                                 bias=nmr[:, t, :],
                                 scale=rstd[:, t, :])
            if g < 4:
                # [tok, 1024] -> [ch%128, ch//128, tok] on the XBAR
                xsT = ph1t.tile([128, KT, 128], BF16, name="xsT")
                nc.sync.dma_start_transpose(out=xsT, in_=xg[:, t, :])
                xsTs.append(xsT)
        return xg, xsTs

    def phase1_compute(g, pre):
        xg, xsTs = pre
        if g >= 4:
            # groups under the in-flight AllReduce: XBAR DMAs freeze during
            # collectives, so transpose on the PE instead (bf16, 1 cyc/row)
            xsTs = []
            for t in range(GB):
                pxT = ph1tp.tile([128, KT, 128], BF16, name="pxT")
                for kt in range(KT):
                    nc.tensor.transpose(pxT[:, kt, :],
                                        xg[:, t, kt * 128:(kt + 1) * 128],
                                        ident)
                xsT = ph1t.tile([128, KT, 128], BF16, name="xsT")
                nc.vector.tensor_copy(out=xsT, in_=pxT)
                xsTs.append(xsT)
        for t in range(GB):
            tt = g * GB + t
            ch = tt // TCH
            ti = tt % TCH
            xsT = xsTs[t]
            pqkv = ph1q.tile([128, QKVW], F32, name="pqkv")
            for kt in range(KT):
                nc.tensor.matmul(
                    pqkv,
                    lhsT=xsT[:, kt, :],
                    rhs=w_all_sb[:, kt, :],
                    start=(kt == 0), stop=False)
            # bias (and bias-sum stat constants) as a rank-1 accumulation
            nc.tensor.matmul(pqkv, lhsT=ones_1p, rhs=b_row,
                             start=False, stop=True)

            # evictions (PSUM fp32 -> SBUF bf16/fp8); bias already added
            nc.scalar.copy(out=qk_pre[:, tt, :], in_=pqkv[:, 0:256])
            nc.vector.tensor_copy(
                out=v_aug[:, tt, :].rearrange("p (h e) -> p h e", e=72)[:, :, 0:64],
                in_=pqkv[:, 256:384].rearrange("p (h e) -> p h e", e=64))
            # q/k sums rode the matmul in the 2 extra weight columns
            nc.vector.tensor_copy(out=stats[:, ch, ti, 0:2],
                                  in_=pqkv[:, QKV:QKV + 2])
            # sums of squares on the otherwise idle Pool engine
            sq = ph1s.tile([128, 2, 128], F32, name="sq")
            nc.gpsimd.tensor_tensor(
                out=sq.rearrange("p a b -> p (a b)"), in0=qk_pre[:, tt, :],
                in1=qk_pre[:, tt, :], op=ALU.mult)
            nc.vector.tensor_reduce(out=stats[:, ch, ti, 2:4], in_=sq,
                                    axis=AX.X, op=ALU.add)

    def emit_allreduce(ch):
        nc.scalar.dma_start(out=cc_in[ch],
                            in_=stats[:, ch].rearrange("p a b -> p (a b)"))
        nc.gpsimd.collective_compute(
            "AllReduce", ALU.add,
            replica_groups=[list(range(n_cores))],
            ins=[cc_in[ch].opt()], outs=[cc_out[ch].opt()])
        nc.sync.dma_start(
            out=stats_all[:, ch].rearrange("p a b -> p (a b)"),
            in_=cc_out[ch])

    # phase 2+3 for one chunk: full-inner LN stats -> normalize -> transpose
    def phase23_chunk(ch):
        qk_sn = []
        for which in range(2):  # 0 -> q, 1 -> k
            s_sum = stats_all[:, ch, :, which]
            s_ssq = stats_all[:, ch, :, 2 + which]
            m = ph2.tile([128, TCH], F32, name=f"m_{ch}_{which}")
            nc.vector.tensor_scalar(out=m, in0=s_sum, scalar1=1.0 / INNER,
                                    scalar2=None, op0=ALU.mult)
            msq = ph2.tile([128, TCH], F32, name=f"msq_{ch}_{which}")
            nc.vector.tensor_scalar(out=msq, in0=s_ssq, scalar1=1.0 / INNER,
                                    scalar2=None, op0=ALU.mult)
            tmp = ph2.tile([128, TCH], F32, name=f"tmp_{ch}_{which}")
            nc.vector.tensor_tensor(out=tmp, in0=m, in1=m, op=ALU.mult)
            nc.vector.tensor_tensor(out=tmp, in0=msq, in1=tmp, op=ALU.subtract)
            nc.vector.tensor_scalar(out=tmp, in0=tmp, scalar1=EPS,
                                    scalar2=None, op0=ALU.add)
            rstd = _quake_rsqrt(nc, ph2, tmp, (128, TCH),
                                suffix=f"_{ch}_{which}")
            nmr = ph2.tile([128, TCH], F32, name=f"nmr_{ch}_{which}")
            nc.vector.tensor_tensor(out=nmr, in0=m, in1=rstd, op=ALU.mult)
            nc.vector.tensor_scalar(out=nmr, in0=nmr, scalar1=-1.0,
                                    scalar2=None, op0=ALU.mult)
            qk_sn.append((m, rstd, nmr))

        (mq, rq, nq) = qk_sn[0]
        (mk, rk, nk) = qk_sn[1]
        # normalize into one [128, TCH, 128] staging tile per tensor, then a
        # SINGLE chunk-wide XBAR transpose each ([tok, (tile col)] ->
        # [col, (tile tok)]), then one chunk-wide gain pass
        qnc = ph3.tile([128, TCH, 128], BF16, name="qnc")
        knc = ph3.tile([128, TCH, 128], BF16, name="knc")
        for ti in range(TCH):
            tt = ch * TCH + ti
            nc.vector.tensor_scalar(
                out=qnc[:, ti, :], in0=qk_pre[:, tt, 0:128],
                scalar1=mq[:, ti:ti + 1], scalar2=rq[:, ti:ti + 1],
                op0=ALU.subtract, op1=ALU.mult)
            nc.scalar.activation(
                out=knc[:, ti, :], in_=qk_pre[:, tt, 128:256],
                func=AF.Identity,
                bias=nk[:, ti:ti + 1], scale=rk[:, ti:ti + 1])
        lo, hi = ch * TCH * 128, (ch + 1) * TCH * 128
        teng = nc.scalar if ch == 0 else nc.sync
        teng.dma_start_transpose(
            out=qT[:, lo:hi].rearrange("p (a b) -> p a b", a=TCH),
            in_=qnc[:, :, :].rearrange("p a b -> p (a b)"))
        teng.dma_start_transpose(
            out=kTt[:, lo:hi].rearrange("p (a b) -> p a b", a=TCH),
            in_=knc[:, :, :].rearrange("p a b -> p (a b)"))
        nc.vector.tensor_scalar(
            out=qT[:, lo:hi], in0=qT[:, lo:hi],
            scalar1=gbe_sb[:, 0:1], scalar2=gbe_sb[:, 1:2],
            op0=ALU.mult, op1=ALU.add)
        nc.scalar.activation(
            out=kTt[:, lo:hi], in_=kTt[:, lo:hi], func=AF.Identity,
            bias=gbe_sb[:, 3:4], scale=gbe_sb[:, 2:3])

    # ---------------- phase 4: attention ----------------
    att = octx.enter_context(tc.tile_pool(name="att", bufs=3))
    dramsc = octx.enter_context(tc.tile_pool(name="dramsc", bufs=2,
                                             space="DRAM"))
    dnp = octx.enter_context(tc.tile_pool(name="dnp", bufs=2))
    actx = ExitStack()           # attention PSUM, closed before phase-5 PSUM

    NPAIR = KB // 2
    DR = mybir.MatmulPerfMode.DoubleRow

    def attention_g(b, h, g, attp, attpo, filler=None):
        # one q-chunk group: tokens [g*1024, (g+1)*1024) of batch b, head h.
        # filler: list of thunks emitting one PE op each (out-projection
        # matmuls), drained one per kb to fill the exp-wait bubbles.
        slot = b * 4 + h * 2 + g

        def emit_pv(pO, eS2, pair):
            # fp8 DoubleRow PV: contract 256 keys (2 kb tiles) per matmul
            # at 0.5 cycles/row
            vt0 = b * KB + 2 * pair
            for qi in range(2):
                nc.tensor.matmul(
                    pO[0:65, qi * 512:(qi + 1) * 512],
                    lhsT=v_aug[:, vt0:vt0 + 2, h * 72:h * 72 + 65],
                    rhs=eS2[:, :, qi * 512:(qi + 1) * 512],
                    start=(pair == 0), stop=(pair == NPAIR - 1),
                    perf_mode=DR)

        pO = attpo.tile([128, 1024], F32, name="pO", tag="pO")
        pend = None          # software pipeline: delay PV by one kb pair
        eS2 = None
        for kb in range(KB):
            pS = attp.tile([128, 1024], F32, name="pS", tag="pS")
            for qi in range(2):
                q0 = b * N + g * 1024 + qi * 512
                nc.tensor.matmul(
                    pS[:, qi * 512:(qi + 1) * 512],
                    lhsT=kTt[h * 64:(h + 1) * 64,
                             b * N + kb * 128:b * N + (kb + 1) * 128],
                    rhs=qT[h * 64:(h + 1) * 64, q0:q0 + 512],
                    start=True, stop=True)
            if pend is not None and kb % 2 == 0:
                emit_pv(pO, *pend)
                pend = None
            if filler:
                filler.pop(0)()
            if kb % 2 == 0:
                eS2 = att.tile([128, 2, 1024], FP8, name="eS2")
            nc.scalar.activation(out=eS2[:, kb % 2, :], in_=pS,
                                 func=AF.Exp)
            if kb % 2 == 1:
                pend = (eS2, kb // 2)
        emit_pv(pO, *pend)
        # evict unnormalized O + raw denominator row
        nc.vector.tensor_copy(out=o_un[0:65, slot, :], in_=pO[0:65, :])

    def denorm_batch(b):
        # batch b's denominators live in o_un[64, b*4:(b+1)*4, :]
        dn_dram = dramsc.tile([1, 4096], BF16, name="dn_dram")
        nc.sync.dma_start(
            out=dn_dram,
            in_=o_un[64:65, b * 4:(b + 1) * 4, :].rearrange(
                "p a t -> p (a t)"))
        dn_g = dnp.tile([128, 32], BF16, name="dn_g")
        nc.sync.dma_start(
            out=dn_g,
            in_=dn_dram[0:1, :].rearrange("o (p c) -> (o p) c", p=128))
        rdn = dnp.tile([128, 32], BF16, name="rdn")
        with nc.allow_low_precision(reason="softmax denom reciprocal, 2e-2 budget"):
            nc.vector.reciprocal(out=rdn, in_=dn_g)
        rdn_dram = dramsc.tile([1, 4096], BF16, name="rdn_dram")
        nc.sync.dma_start(
            out=rdn_dram[0:1, :].rearrange("o (p c) -> (o p) c", p=128),
            in_=rdn)
        dnb = dnp.tile([64, 4096], BF16, name="dnb")
        nc.sync.dma_start(out=dnb, in_=rdn_dram.to_broadcast([64, 4096]))
        for h in range(HL):
            for g in range(2):
                slot = b * 4 + h * 2 + g
                sg = h * 2 + g
                nc.vector.tensor_tensor(
                    out=onorm[h * 64:(h + 1) * 64,
                              b * N + g * 1024:b * N + (g + 1) * 1024],
                    in0=o_un[0:64, slot, :],
                    in1=dnb[:, sg * 1024:(sg + 1) * 1024],
                    op=ALU.mult)

    def silu_batch(b):
        nc.scalar.activation(out=siluo[:, b * N:(b + 1) * N],
                             in_=onorm[:, b * N:(b + 1) * N], func=AF.Silu)

    # ---------------- emission schedule ----------------
    pre = phase1_stats(0)
    for g in range(NG):
        nxt = phase1_stats(g + 1) if g + 1 < NG else None
        phase1_compute(g, pre)
        pre = nxt
        if g == 3:
            emit_allreduce(0)
    # ph23(0)'s XBAR transposes freeze while a collective is in flight, so
    # launch the second AllReduce only after they are issued
    phase23_chunk(0)
    emit_allreduce(1)
    pctx.close()                 # free phase-1 PSUM banks
    attp = actx.enter_context(tc.tile_pool(name="attp", bufs=2, space="PSUM"))
    attpo = actx.enter_context(tc.tile_pool(name="attpo", bufs=1,
                                            space="PSUM"))
    ph5p = actx.enter_context(tc.tile_pool(name="ph5p", bufs=2, space="PSUM"))
    ph5 = octx.enter_context(tc.tile_pool(name="ph5", bufs=4))

    def make_ph5_op(tk, ct, on_act=False):
        def emit():
            po = ph5p.tile([128, 512], F32, name="po")
            nc.tensor.matmul(
                po,
                lhsT=w_o_sb[:, ct * 128:(ct + 1) * 128],
                rhs=siluo[:, tk * 512:(tk + 1) * 512],
                start=True, stop=True)
            ev = ph5.tile([128, 512], BF16, name="ev")
            if on_act:
                nc.scalar.copy(out=ev, in_=po)
            else:
                nc.vector.tensor_copy(out=ev, in_=po)
            nc.sync.dma_start(
                out=out_t[ct * 128:(ct + 1) * 128,
                          tk * 512:(tk + 1) * 512],
                in_=ev)
        return emit

    attention_g(0, 0, 0, attp, attpo)
    attention_g(0, 0, 1, attp, attpo)
    attention_g(0, 1, 0, attp, attpo)
    phase23_chunk(1)
    attention_g(0, 1, 1, attp, attpo)
    denorm_batch(0)
    attention_g(1, 0, 0, attp, attpo)
    attention_g(1, 0, 1, attp, attpo)
    silu_batch(0)
    # batch-0 out-projection interleaved into batch-1 attention: one matmul
    # per kb fills the PE bubble left by the exp dependency
    fill = [make_ph5_op(tk, ct) for tk in range(4) for ct in range(KT)]
    attention_g(1, 1, 0, attp, attpo, filler=fill)
    attention_g(1, 1, 1, attp, attpo, filler=fill)
    assert not fill
    denorm_batch(1)
    silu_batch(1)
    for tk in range(4, 8):
        for ct in range(KT):
            make_ph5_op(tk, ct, on_act=(ct % 2 == 1))()
    actx.close()                 # free attention PSUM banks

    octx.close()


def make_in_maps(inputs, n_tok_per_batch, n_cores=NCORES):
    """Slice full inputs into per-core input maps (head sharding)."""
    import ml_dtypes
    bf16 = ml_dtypes.bfloat16

    x = np.ascontiguousarray(np.asarray(inputs["x"], np.float32)
                             .reshape(B * n_tok_per_batch, C)).astype(bf16)
    w_q = np.asarray(inputs["w_q"], np.float32)
    w_k = np.asarray(inputs["w_k"], np.float32)
    w_v = np.asarray(inputs["w_v"], np.float32)
    b_q = np.asarray(inputs["b_q"], np.float32)
    b_k = np.asarray(inputs["b_k"], np.float32)
    b_v = np.asarray(inputs["b_v"], np.float32)
    g_q = np.asarray(inputs["g_q"], np.float32)
    be_q = np.asarray(inputs["be_q"], np.float32)
    g_k = np.asarray(inputs["g_k"], np.float32)
    be_k = np.asarray(inputs["be_k"], np.float32)
    w_o = np.asarray(inputs["w_o"], np.float32)

    scale = float(INNER) ** -0.5
    in_maps = []
    for c in range(n_cores):
        cols = slice(c * CL, (c + 1) * CL)
        wq_l = w_q[:, cols]
        wk_l = w_k[:, cols]
        wv_l = w_v[:, cols]
        w_all = np.ascontiguousarray(np.concatenate(
            [wq_l, wk_l, wv_l,
             wq_l.sum(axis=1, keepdims=True),
             wk_l.sum(axis=1, keepdims=True)], axis=1)).astype(bf16)
        b_all = np.ascontiguousarray(
            np.concatenate([b_q[cols], b_k[cols], b_v[cols],
                            [b_q[cols].sum()], [b_k[cols].sum()]])[None, :]
        ).astype(bf16)
        gbe = np.ascontiguousarray(
            np.stack([g_q[cols] * scale, be_q[cols] * scale,
                      g_k[cols], be_k[cols]], axis=1))
        w_o_c = np.ascontiguousarray(w_o[cols, :]).astype(bf16)
        in_maps.append({
            "x": x, "w_all": w_all, "b_all": b_all,
            "gbe": gbe, "w_o_loc": w_o_c,
        })
    return in_maps


def combine_outputs(out_ts, inputs, n_tok_per_batch):
    b_o = np.asarray(inputs["b_o"], np.float32)
    acc = np.zeros(out_ts[0].shape, dtype=np.float32)
    for o in out_ts:
        acc += np.asarray(o, dtype=np.float32)
    out = acc.T + b_o[None, :]
    return out.reshape(B, n_tok_per_batch, C).astype(np.float32)


_NC_CACHE = {}


def kernel(**inputs):
    from concourse.bass_utils import run_bass_kernel_spmd

    n_tok = np.asarray(inputs["x"]).shape[1]
    if n_tok not in _NC_CACHE:
        _NC_CACHE[n_tok] = build_bass(n_tok)
    nc = _NC_CACHE[n_tok]
    in_maps = make_in_maps(inputs, n_tok)
    res = run_bass_kernel_spmd(nc, in_maps, core_ids=list(range(NCORES)))
    out_ts = [r["out_t"] for r in res.results]
    return combine_outputs(out_ts, inputs, n_tok)


# revision 34
# speedup vs baseline: 1.0828x; 1.0168x over previous
"""Trainium2 Bass kernel for nn_Attention_71846212928150.

Self-attention block (pre-LN + silu, QKV projections, per-head attention with
q/k LayerNorms, output projection), sharded over 8 NeuronCores by heads:
core c owns heads {2c, 2c+1} = inner columns [128c, 128c+128).

v2 design (vs. the fp32r v1): all PE operands are bf16 (PSUM accumulation
stays fp32), all transposes run on the DMA XBAR (dma_start_transpose) instead
of the PE, q/k LN sums ride the QKV matmul as two host-precomputed row-sum
weight columns, the stats AllReduce is split into two chunks overlapped with
compute, the attention loop is software-pipelined (S(kb+1) issued before
PV(kb)) with double-buffered PSUM so the PE never idles, and the softmax
denominators are batched into a [128, 32] reciprocal instead of a 1-partition
15us DVE reciprocal per (batch, head).
"""

import numpy as np

import concourse.bass as bass
import concourse.mybir as mybir
import concourse.tile as tile
from concourse.masks import make_identity

F32 = mybir.dt.float32
BF16 = mybir.dt.bfloat16
FP8 = mybir.dt.float8e4
I32 = mybir.dt.int32
AF = mybir.ActivationFunctionType
ALU = mybir.AluOpType
AX = mybir.AxisListType

B = 2
C = 1024
H = 16
DH = 64
INNER = H * DH
NCORES = 8
HL = H // NCORES          # 2 heads per core
CL = HL * DH              # 128 local inner columns
QKV = 3 * CL              # 384
QKVW = QKV + 2            # + sum_q / sum_k stat columns
KT = C // 128             # 8 contraction tiles over C
EPS = 1e-5
MAGIC = 0x5F3759DF


def _quake_rsqrt(nc, pool, vpe, shape, iters=3, suffix=""):
    """rstd = 1/sqrt(vpe) entirely on DVE (fp32 bitcast + Newton steps)."""
    y = pool.tile(list(shape), F32, name=f"qk_y{suffix}")
    t2 = pool.tile(list(shape), F32, name=f"qk_t2{suffix}")
    nc.vector.tensor_scalar(
        out=y.bitcast(I32), in0=vpe.bitcast(I32), scalar1=1, scalar2=None,
        op0=ALU.logical_shift_right)
    nc.vector.tensor_scalar(
        out=y.bitcast(I32), in0=y.bitcast(I32), scalar1=-1, scalar2=MAGIC,
        op0=ALU.mult, op1=ALU.add)
    for _ in range(iters):
        nc.vector.tensor_tensor(out=t2, in0=y, in1=y, op=ALU.mult)
        nc.vector.tensor_tensor(out=t2, in0=t2, in1=vpe, op=ALU.mult)
        nc.vector.tensor_scalar(out=t2, in0=t2, scalar1=-0.5, scalar2=1.5,
                                op0=ALU.mult, op1=ALU.add)
        nc.vector.tensor_tensor(out=y, in0=y, in1=t2, op=ALU.mult)
    return y


def _fixup_module(nc):
    """Adapt Tile-emitted BIR to this container's walrus build.

    1. The tail `EVENT_SEMAPHORE_RANGE_CLEAR` InstISA (opcode 176) is not
       understood by this walrus' birverifier. Replace it with one
       EventSemaphore sem-write-0 per semaphore in the cleared range.
    2. Drain instructions carrying more than one semaphore wait fail codegen;
       hoist the extra waits into standalone EventSemaphore waits.
    """
    for f in nc.m.functions:
        for bb in f.blocks:
            newlist = []
            changed = False
            for ins in bb.instructions:
                tn = type(ins).__name__
                if tn == "InstISA" and getattr(ins, "isa_opcode", None) == 176:
                    ad = ins.ant_dict or {}
                    first = ad.get("range_first")
                    last = ad.get("range_last")
                    if first is not None and last is not None:
                        si = ins.sync_info
                        sems = list(range(first, last + 1))
                        for k, sem in enumerate(sems):
                            ev = mybir.InstEventSemaphore(
                                name=f"{ins.name}-clr{k}", engine=ins.engine,
                                ins=[], outs=[])
                            upd = mybir.SyncUpdate(
                                sync_type="semaphore", id=sem,
                                update_mode="sem-wr-imm", update_value=0)
                            on_wait = (list(si.on_wait)
                                       if (k == 0 and si is not None and si.on_wait)
                                       else [])
                            ev.sync_info = mybir.SyncInfo(
                                on_wait=on_wait, on_update=[upd])
                            newlist.append(ev)
                        if si is not None and si.on_update:
                            evf = mybir.InstEventSemaphore(
                                name=f"{ins.name}-clrf", engine=ins.engine,
                                ins=[], outs=[])
                            evf.sync_info = mybir.SyncInfo(
                                on_wait=[], on_update=list(si.on_update))
                            newlist.append(evf)
                    changed = True
                    continue
                si = ins.sync_info
                if (si is not None and si.on_wait is not None
                        and len(si.on_wait) > 1):
                    waits = list(si.on_wait)
                    for i, w in enumerate(waits[1:]):
                        ev = mybir.InstEventSemaphore(
                            name=f"{ins.name}-hw{i}", engine=ins.engine,
                            ins=[], outs=[])
                        ev.sync_info = mybir.SyncInfo(on_wait=[w], on_update=[])
                        newlist.append(ev)
                    si.on_wait = [waits[0]]
                    ins.sync_info = si
                    changed = True
                newlist.append(ins)
            if changed:
                bb.instructions = newlist
    return nc


def build_bass(n_tok_per_batch, n_cores=NCORES):
    N = n_tok_per_batch
    T = B * N
    NT = T // 128             # token tiles (32)
    KB = N // 128             # key tiles per batch (16)

    nc = bass.Bass(trn_type="TRN2", num_devices=n_cores)

    x = nc.dram_tensor("x", [T, C], BF16, kind="ExternalInput")
    w_all = nc.dram_tensor("w_all", [C, QKVW], BF16, kind="ExternalInput")
    b_all = nc.dram_tensor("b_all", [1, QKVW], BF16, kind="ExternalInput")
    gbe = nc.dram_tensor("gbe", [128, 4], F32, kind="ExternalInput")
    w_o_loc = nc.dram_tensor("w_o_loc", [CL, C], BF16, kind="ExternalInput")
    out_t = nc.dram_tensor("out_t", [C, T], BF16, kind="ExternalOutput")

    with tile.TileContext(nc) as tc:
        _body(tc, x, w_all, b_all, gbe, w_o_loc, out_t,
              N=N, T=T, NT=NT, KB=KB, n_cores=n_cores)
    return _fixup_module(nc)


def _body(tc, x, w_all, b_all, gbe, w_o_loc, out_t, N, T, NT, KB, n_cores):
    nc = tc.nc

    from contextlib import ExitStack
    octx = ExitStack()
    persist = octx.enter_context(tc.tile_pool(name="persist", bufs=1))

    GB = 4                       # token tiles per phase-1 group
    NG = NT // GB                # 8 groups
    NCH = 2                      # AllReduce chunks (chunk == batch)
    TCH = NT // NCH              # 16 tiles per chunk

    ident = persist.tile([128, 128], BF16)
    make_identity(nc, ident)

    w_all_sb = persist.tile([128, KT, QKVW], BF16)
    for kt in range(KT):
        nc.scalar.dma_start(out=w_all_sb[:, kt, :],
                          in_=w_all[kt * 128:(kt + 1) * 128, :])
    b_row = persist.tile([1, QKVW], BF16)
    nc.scalar.dma_start(out=b_row, in_=b_all[0:1, :])
    ones_1p = persist.tile([1, 128], BF16)
    nc.vector.memset(ones_1p, 1.0)
    gbe_sb = persist.tile([128, 4], F32)
    nc.scalar.dma_start(out=gbe_sb, in_=gbe[:, :])
    w_o_sb = persist.tile([128, C], BF16)
    nc.scalar.dma_start(out=w_o_sb, in_=w_o_loc[:, :])

    qT = persist.tile([128, T], BF16)       # [local col, token]
    kTt = persist.tile([128, T], BF16)
    v_aug = persist.tile([128, NT, 144], FP8)  # [tok%128, tile, 2x(64 v + 1 + pad)]
    qk_pre = persist.tile([128, NT, 256], BF16)  # [tok%128, tile, q|k col]
    # stats cols: 0=sum_q, 1=sum_k, 2=ssq_q, 3=ssq_k
    stats = persist.tile([128, NCH, TCH, 4], F32)
    stats_all = persist.tile([128, NCH, TCH, 4], F32)
    o_un = persist.tile([128, 2 * B * HL, 1024], BF16)  # [dim(65), slot, qtok]
    onorm = persist.tile([128, T], BF16)
    siluo = persist.tile([128, T], BF16)
    scr = persist.tile([128, 128], BF16)

    ones_col = persist.tile([128, NT], F32)
    nc.vector.memset(ones_col, 1.0)
    nc.vector.tensor_copy(out=v_aug[:, :, 64:65], in_=ones_col)
    nc.vector.tensor_copy(out=v_aug[:, :, 136:137], in_=ones_col)

    dram = octx.enter_context(tc.tile_pool(name="dram", bufs=1, space="DRAM"))
    cc_in = [dram.tile([128, TCH * 4], F32, name=f"cc_in{c}")
             for c in range(NCH)]
    cc_out = [dram.tile([128, TCH * 4], F32, name=f"cc_out{c}",
                        addr_space="Shared")
              for c in range(NCH)]

    ph1 = octx.enter_context(tc.tile_pool(name="ph1", bufs=3))
    ph1t = octx.enter_context(tc.tile_pool(name="ph1t", bufs=8))
    ph1s = octx.enter_context(tc.tile_pool(name="ph1s", bufs=4))
    ph2 = octx.enter_context(tc.tile_pool(name="ph2", bufs=1))
    ph3 = octx.enter_context(tc.tile_pool(name="ph3", bufs=2))

    pctx = ExitStack()           # phase-1 PSUM, closed before attention PSUM
    ph1q = pctx.enter_context(tc.tile_pool(name="ph1q", bufs=3, space="PSUM"))
    ph1tp = pctx.enter_context(tc.tile_pool(name="ph1tp", bufs=2, space="PSUM"))

    # ---------------- phase 1: x LN+silu, XBAR transpose, QKV ----------------
    # Split into a stats part (no PE dependency) and a compute part, emitted
    # one group ahead, so the DVE queue prefetches bn_stats instead of
    # head-of-line blocking on matmul-dependent evictions.
    def phase1_stats(g):
        xg = ph1.tile([128, GB, C], BF16, name="xg")
        nc.scalar.dma_start(
            out=xg,
            in_=x[g * GB * 128:(g + 1) * GB * 128, :].rearrange(
                "(t p) c -> p t c", p=128))

        stats6 = ph1s.tile([128, GB, 2, 6], F32, name="stats6")
        for t in range(GB):
            for h2 in range(2):
                nc.vector.bn_stats(out=stats6[:, t, h2, :],
                                   in_=xg[:, t, h2 * 512:(h2 + 1) * 512])
        mv = ph1s.tile([128, GB, 2], F32, name="mv")
        for t in range(GB):
            nc.vector.bn_aggr(out=mv[:, t, :], in_=stats6[:, t, :, :])

        vpe = ph1s.tile([128, GB, 1], F32, name="vpe")
        nc.vector.tensor_scalar(out=vpe, in0=mv[:, :, 1:2], scalar1=EPS,
                                scalar2=None, op0=ALU.add)
        rstd = _quake_rsqrt(nc, ph1s, vpe, (128, GB, 1), iters=2, suffix="x")
        nmr = ph1s.tile([128, GB, 1], F32, name="nmr")
        nc.vector.tensor_tensor(out=nmr, in0=mv[:, :, 0:1], in1=rstd,
                                op=ALU.mult)
        nc.vector.tensor_scalar(out=nmr, in0=nmr, scalar1=-1.0,
                                scalar2=None, op0=ALU.mult)
        # silu(LN(x)) + x^T XBAR here (no PE dependency), so the next group's
        # transposed input is ready before this group's matmuls retire and the
        # PE never waits at a group boundary
        xsTs = []
        for t in range(GB):
            nc.scalar.activation(out=xg[:, t, :], in_=xg[:, t, :],
                                 func=AF.Silu,
                                 bias=nmr[:, t, :],
                                 scale=rstd[:, t, :])
        return xg, xsTs

    def phase1_compute(g, pre):
        xg, xsTs = pre
        if True:
            # transpose on the PE (bf16, 1 cyc/row): XBAR DMAs freeze during
            # in-flight collectives and add latency to the group pipeline
            xsTs = []
            for t in range(GB):
                pxT = ph1tp.tile([128, KT, 128], BF16, name="pxT")
                for kt in range(KT):
                    nc.tensor.transpose(pxT[:, kt, :],
                                        xg[:, t, kt * 128:(kt + 1) * 128],
                                        ident)
                xsT = ph1t.tile([128, KT, 128], BF16, name="xsT")
                nc.vector.tensor_copy(out=xsT, in_=pxT)
                xsTs.append(xsT)
        for t in range(GB):
            tt = g * GB + t
            ch = tt // TCH
            ti = tt % TCH
            xsT = xsTs[t]
            pqkv = ph1q.tile([128, QKVW], F32, name="pqkv")
            for kt in range(KT):
                nc.tensor.matmul(
                    pqkv,
                    lhsT=xsT[:, kt, :],
                    rhs=w_all_sb[:, kt, :],
                    start=(kt == 0), stop=False)
            # bias (and bias-sum stat constants) as a rank-1 accumulation
            nc.tensor.matmul(pqkv, lhsT=ones_1p, rhs=b_row,
                             start=False, stop=True)

            # evictions (PSUM fp32 -> SBUF bf16/fp8); bias already added
            nc.scalar.copy(out=qk_pre[:, tt, :], in_=pqkv[:, 0:256])
            nc.vector.tensor_copy(
                out=v_aug[:, tt, :].rearrange("p (h e) -> p h e", e=72)[:, :, 0:64],
                in_=pqkv[:, 256:384].rearrange("p (h e) -> p h e", e=64))
            # q/k sums rode the matmul in the 2 extra weight columns
            nc.vector.tensor_copy(out=stats[:, ch, ti, 0:2],
                                  in_=pqkv[:, QKV:QKV + 2])
            # sums of squares on the otherwise idle Pool engine
            sq = ph1s.tile([128, 2, 128], F32, name="sq")
            nc.gpsimd.tensor_tensor(
                out=sq.rearrange("p a b -> p (a b)"), in0=qk_pre[:, tt, :],
                in1=qk_pre[:, tt, :], op=ALU.mult)
            nc.vector.tensor_reduce(out=stats[:, ch, ti, 2:4], in_=sq,
                                    axis=AX.X, op=ALU.add)

    def emit_allreduce(ch):
        nc.scalar.dma_start(out=cc_in[ch],
                            in_=stats[:, ch].rearrange("p a b -> p (a b)"))
        nc.gpsimd.collective_compute(
            "AllReduce", ALU.add,
            replica_groups=[list(range(n_cores))],
            ins=[cc_in[ch].opt()], outs=[cc_out[ch].opt()])
        nc.sync.dma_start(
            out=stats_all[:, ch].rearrange("p a b -> p (a b)"),
            in_=cc_out[ch])

    # phase 2+3 for one chunk: full-inner LN stats -> normalize -> transpose
    def phase23_chunk(ch):
        qk_sn = []
        for which in range(2):  # 0 -> q, 1 -> k
            s_sum = stats_all[:, ch, :, which]
            s_ssq = stats_all[:, ch, :, 2 + which]
            m = ph2.tile([128, TCH], F32, name=f"m_{ch}_{which}")
            nc.vector.tensor_scalar(out=m, in0=s_sum, scalar1=1.0 / INNER,
                                    scalar2=None, op0=ALU.mult)
            msq = ph2.tile([128, TCH], F32, name=f"msq_{ch}_{which}")
            nc.vector.tensor_scalar(out=msq, in0=s_ssq, scalar1=1.0 / INNER,
                                    scalar2=None, op0=ALU.mult)
            tmp = ph2.tile([128, TCH], F32, name=f"tmp_{ch}_{which}")
            nc.vector.tensor_tensor(out=tmp, in0=m, in1=m, op=ALU.mult)
            nc.vector.tensor_tensor(out=tmp, in0=msq, in1=tmp, op=ALU.subtract)
            nc.vector.tensor_scalar(out=tmp, in0=tmp, scalar1=EPS,
                                    scalar2=None, op0=ALU.add)
            rstd = _quake_rsqrt(nc, ph2, tmp, (128, TCH),
                                suffix=f"_{ch}_{which}")
            nmr = ph2.tile([128, TCH], F32, name=f"nmr_{ch}_{which}")
            nc.vector.tensor_tensor(out=nmr, in0=m, in1=rstd, op=ALU.mult)
            nc.vector.tensor_scalar(out=nmr, in0=nmr, scalar1=-1.0,
                                    scalar2=None, op0=ALU.mult)
            qk_sn.append((m, rstd, nmr))

        (mq, rq, nq) = qk_sn[0]
        (mk, rk, nk) = qk_sn[1]
        # normalize into one [128, TCH, 128] staging tile per tensor, then a
        # SINGLE chunk-wide XBAR transpose each ([tok, (tile col)] ->
        # [col, (tile tok)]), then one chunk-wide gain pass
        qnc = ph3.tile([128, TCH, 128], BF16, name="qnc")
        knc = ph3.tile([128, TCH, 128], BF16, name="knc")
        for ti in range(TCH):
            tt = ch * TCH + ti
            nc.vector.tensor_scalar(
                out=qnc[:, ti, :], in0=qk_pre[:, tt, 0:128],
                scalar1=mq[:, ti:ti + 1], scalar2=rq[:, ti:ti + 1],
                op0=ALU.subtract, op1=ALU.mult)
            nc.scalar.activation(
                out=knc[:, ti, :], in_=qk_pre[:, tt, 128:256],
                func=AF.Identity,
                bias=nk[:, ti:ti + 1], scale=rk[:, ti:ti + 1])
        lo, hi = ch * TCH * 128, (ch + 1) * TCH * 128
        teng = nc.scalar if ch == 0 else nc.sync
        teng.dma_start_transpose(
            out=qT[:, lo:hi].rearrange("p (a b) -> p a b", a=TCH),
            in_=qnc[:, :, :].rearrange("p a b -> p (a b)"))
        teng.dma_start_transpose(
            out=kTt[:, lo:hi].rearrange("p (a b) -> p a b", a=TCH),
            in_=knc[:, :, :].rearrange("p a b -> p (a b)"))
        nc.vector.tensor_scalar(
            out=qT[:, lo:hi], in0=qT[:, lo:hi],
            scalar1=gbe_sb[:, 0:1], scalar2=gbe_sb[:, 1:2],
            op0=ALU.mult, op1=ALU.add)
        nc.scalar.activation(
            out=kTt[:, lo:hi], in_=kTt[:, lo:hi], func=AF.Identity,
            bias=gbe_sb[:, 3:4], scale=gbe_sb[:, 2:3])

    # ---------------- phase 4: attention ----------------
    att = octx.enter_context(tc.tile_pool(name="att", bufs=3))
    dramsc = octx.enter_context(tc.tile_pool(name="dramsc", bufs=2,
                                             space="DRAM"))
    dnp = octx.enter_context(tc.tile_pool(name="dnp", bufs=2))
    actx = ExitStack()           # attention PSUM, closed before phase-5 PSUM

    NPAIR = KB // 2
    DR = mybir.MatmulPerfMode.DoubleRow

    def attention_g(b, h, g, attp, attpo, filler=None):
        # one q-chunk group: tokens [g*1024, (g+1)*1024) of batch b, head h.
        # filler: list of thunks emitting one PE op each (out-projection
        # matmuls), drained one per kb to fill the exp-wait bubbles.
        slot = b * 4 + h * 2 + g

        def emit_pv(pO, eS2, pair):
            # fp8 DoubleRow PV: contract 256 keys (2 kb tiles) per matmul
            # at 0.5 cycles/row
            vt0 = b * KB + 2 * pair
            for qi in range(2):
                nc.tensor.matmul(
                    pO[0:65, qi * 512:(qi + 1) * 512],
                    lhsT=v_aug[:, vt0:vt0 + 2, h * 72:h * 72 + 65],
                    rhs=eS2[:, :, qi * 512:(qi + 1) * 512],
                    start=(pair == 0), stop=(pair == NPAIR - 1),
                    perf_mode=DR)

        pO = attpo.tile([128, 1024], F32, name="pO", tag="pO")
        pend = None          # software pipeline: delay PV by one kb pair
        eS2 = None
        for kb in range(KB):
            pS = attp.tile([128, 1024], F32, name="pS", tag="pS")
            for qi in range(2):
                q0 = b * N + g * 1024 + qi * 512
                nc.tensor.matmul(
                    pS[:, qi * 512:(qi + 1) * 512],
                    lhsT=kTt[h * 64:(h + 1) * 64,
                             b * N + kb * 128:b * N + (kb + 1) * 128],
                    rhs=qT[h * 64:(h + 1) * 64, q0:q0 + 512],
                    start=True, stop=True)
            if pend is not None and kb % 2 == 0:
                emit_pv(pO, *pend)
                pend = None
            if filler:
                filler.pop(0)()
            if kb % 2 == 0:
                eS2 = att.tile([128, 2, 1024], FP8, name="eS2")
            nc.scalar.activation(out=eS2[:, kb % 2, :], in_=pS,
                                 func=AF.Exp)
            if kb % 2 == 1:
                pend = (eS2, kb // 2)
        emit_pv(pO, *pend)
        # evict unnormalized O + raw denominator row
        nc.vector.tensor_copy(out=o_un[0:65, slot, :], in_=pO[0:65, :])

    def denorm_batch(b):
        # batch b's denominators live in o_un[64, b*4:(b+1)*4, :]
        dn_dram = dramsc.tile([1, 4096], BF16, name="dn_dram")
        nc.sync.dma_start(
            out=dn_dram,
            in_=o_un[64:65, b * 4:(b + 1) * 4, :].rearrange(
                "p a t -> p (a t)"))
        dn_g = dnp.tile([128, 32], BF16, name="dn_g")
        nc.sync.dma_start(
            out=dn_g,
            in_=dn_dram[0:1, :].rearrange("o (p c) -> (o p) c", p=128))
        rdn = dnp.tile([128, 32], BF16, name="rdn")
        with nc.allow_low_precision(reason="softmax denom reciprocal, 2e-2 budget"):
            nc.vector.reciprocal(out=rdn, in_=dn_g)
        rdn_dram = dramsc.tile([1, 4096], BF16, name="rdn_dram")
        nc.sync.dma_start(
            out=rdn_dram[0:1, :].rearrange("o (p c) -> (o p) c", p=128),
            in_=rdn)
        dnb = dnp.tile([64, 4096], BF16, name="dnb")
        nc.sync.dma_start(out=dnb, in_=rdn_dram.to_broadcast([64, 4096]))
        for h in range(HL):
            for g in range(2):
                slot = b * 4 + h * 2 + g
                sg = h * 2 + g
                nc.vector.tensor_tensor(
                    out=onorm[h * 64:(h + 1) * 64,
                              b * N + g * 1024:b * N + (g + 1) * 1024],
                    in0=o_un[0:64, slot, :],
                    in1=dnb[:, sg * 1024:(sg + 1) * 1024],
                    op=ALU.mult)

    def silu_batch(b):
        nc.scalar.activation(out=siluo[:, b * N:(b + 1) * N],
                             in_=onorm[:, b * N:(b + 1) * N], func=AF.Silu)

    # ---------------- emission schedule ----------------
    pre = phase1_stats(0)
    for g in range(NG):
        nxt = phase1_stats(g + 1) if g + 1 < NG else None
        phase1_compute(g, pre)
        pre = nxt
        if g == 3:
            emit_allreduce(0)
    # ph23(0)'s XBAR transposes freeze while a collective is in flight, so
    # launch the second AllReduce only after they are issued
    phase23_chunk(0)
    emit_allreduce(1)
    pctx.close()                 # free phase-1 PSUM banks
    attp = actx.enter_context(tc.tile_pool(name="attp", bufs=2, space="PSUM"))
    attpo = actx.enter_context(tc.tile_pool(name="attpo", bufs=1,
                                            space="PSUM"))
    ph5p = actx.enter_context(tc.tile_pool(name="ph5p", bufs=2, space="PSUM"))
    ph5 = octx.enter_context(tc.tile_pool(name="ph5", bufs=4))

    def make_ph5_op(tk, ct, on_act=False):
        def emit():
            po = ph5p.tile([128, 512], F32, name="po")
            nc.tensor.matmul(
                po,
                lhsT=w_o_sb[:, ct * 128:(ct + 1) * 128],
                rhs=siluo[:, tk * 512:(tk + 1) * 512],
                start=True, stop=True)
            ev = ph5.tile([128, 512], BF16, name="ev")
            if on_act:
                nc.scalar.copy(out=ev, in_=po)
            else:
                nc.vector.tensor_copy(out=ev, in_=po)
            nc.sync.dma_start(
                out=out_t[ct * 128:(ct + 1) * 128,
                          tk * 512:(tk + 1) * 512],
                in_=ev)
        return emit

    attention_g(0, 0, 0, attp, attpo)
    attention_g(0, 0, 1, attp, attpo)
    attention_g(0, 1, 0, attp, attpo)
    phase23_chunk(1)
    attention_g(0, 1, 1, attp, attpo)
    denorm_batch(0)
    attention_g(1, 0, 0, attp, attpo)
    attention_g(1, 0, 1, attp, attpo)
    silu_batch(0)
    # batch-0 out-projection interleaved into batch-1 attention: one matmul
    # per kb fills the PE bubble left by the exp dependency
    fill = [make_ph5_op(tk, ct) for tk in range(4) for ct in range(KT)]
    attention_g(1, 1, 0, attp, attpo, filler=fill)
    attention_g(1, 1, 1, attp, attpo, filler=fill)
    assert not fill
    denorm_batch(1)
    silu_batch(1)
    for tk in range(4, 8):
        for ct in range(KT):
            make_ph5_op(tk, ct, on_act=(ct % 2 == 1))()
    actx.close()                 # free attention PSUM banks

    octx.close()


def make_in_maps(inputs, n_tok_per_batch, n_cores=NCORES):
    """Slice full inputs into per-core input maps (head sharding)."""
    import ml_dtypes
    bf16 = ml_dtypes.bfloat16

    x = np.ascontiguousarray(np.asarray(inputs["x"], np.float32)
                             .reshape(B * n_tok_per_batch, C)).astype(bf16)
    w_q = np.asarray(inputs["w_q"], np.float32)
    w_k = np.asarray(inputs["w_k"], np.float32)
    w_v = np.asarray(inputs["w_v"], np.float32)
    b_q = np.asarray(inputs["b_q"], np.float32)
    b_k = np.asarray(inputs["b_k"], np.float32)
    b_v = np.asarray(inputs["b_v"], np.float32)
    g_q = np.asarray(inputs["g_q"], np.float32)
    be_q = np.asarray(inputs["be_q"], np.float32)
    g_k = np.asarray(inputs["g_k"], np.float32)
    be_k = np.asarray(inputs["be_k"], np.float32)
    w_o = np.asarray(inputs["w_o"], np.float32)

    scale = float(INNER) ** -0.5
    in_maps = []
    for c in range(n_cores):
        cols = slice(c * CL, (c + 1) * CL)
        wq_l = w_q[:, cols]
        wk_l = w_k[:, cols]
        wv_l = w_v[:, cols]
        w_all = np.ascontiguousarray(np.concatenate(
            [wq_l, wk_l, wv_l,
             wq_l.sum(axis=1, keepdims=True),
             wk_l.sum(axis=1, keepdims=True)], axis=1)).astype(bf16)
        b_all = np.ascontiguousarray(
            np.concatenate([b_q[cols], b_k[cols], b_v[cols],
                            [b_q[cols].sum()], [b_k[cols].sum()]])[None, :]
        ).astype(bf16)
        gbe = np.ascontiguousarray(
            np.stack([g_q[cols] * scale, be_q[cols] * scale,
                      g_k[cols], be_k[cols]], axis=1))
        w_o_c = np.ascontiguousarray(w_o[cols, :]).astype(bf16)
        in_maps.append({
            "x": x, "w_all": w_all, "b_all": b_all,
            "gbe": gbe, "w_o_loc": w_o_c,
        })
    return in_maps


def combine_outputs(out_ts, inputs, n_tok_per_batch):
    b_o = np.asarray(inputs["b_o"], np.float32)
    acc = np.zeros(out_ts[0].shape, dtype=np.float32)
    for o in out_ts:
        acc += np.asarray(o, dtype=np.float32)
    out = acc.T + b_o[None, :]
    return out.reshape(B, n_tok_per_batch, C).astype(np.float32)


_NC_CACHE = {}


def kernel(**inputs):
    from concourse.bass_utils import run_bass_kernel_spmd

    n_tok = np.asarray(inputs["x"]).shape[1]
    if n_tok not in _NC_CACHE:
        _NC_CACHE[n_tok] = build_bass(n_tok)
    nc = _NC_CACHE[n_tok]
    in_maps = make_in_maps(inputs, n_tok)
    res = run_bass_kernel_spmd(nc, in_maps, core_ids=list(range(NCORES)))
    out_ts = [r["out_t"] for r in res.results]
    return combine_outputs(out_ts, inputs, n_tok)
